# revision 1
# baseline (speedup 1.0000x reference)
"""Global-attention kernel for [8, 384, 32, 32] ConvAttention on 8 trn2 cores.

Math (per reference): tokens over B*H*W = 8192 positions, C = 384 channels
split as V/K/Q of 128 each; out = softmax(Q K^T / sqrt(128)) V, re-laid as
[B, 128, H, W].

Sharding: core c owns the 1024 query tokens of batch c (token n = b*1024+hw,
so batch == contiguous token block). K/V are replicated. Each core computes
its row block of the attention entirely locally; no collectives.

On-core layout: everything channel-major ([d, token]) which is exactly how
x is laid out in DRAM, so host prep is just slicing + two cheap transposes:
  qT [128, 1024]  = x[c, 256:384].reshape(128, 1024)          (per core)
  kT [128, 8192]  = x[:, 128:256] tokens, channel-major        (replicated)
  vt [128, 8192]  = V tokens chunk-transposed: vt[p, 128*j+v] = V[128*j+p, v]
The S^T = K_chunk Q^T matmul then needs no on-chip transposes at all, the
softmax denominator comes from a ones-vector matmul (partition reduction on
PE), and the output lands directly in [v, token] = DRAM layout.
"""

import math

import numpy as np

import concourse.bass as bass
import concourse.tile as tile
from concourse import bacc, mybir
from concourse.bass_utils import run_bass_kernel_spmd

N_CORES = 8
B, C, H, W = 8, 384, 32, 32
HW = H * W            # 1024 tokens per batch == per core
N = B * HW            # 8192 total tokens
D = 128               # key/value width
NCHUNK = N // 128     # 64 kv chunks of 128 tokens
SCALE = 1.0 / math.sqrt(D)
F32 = mybir.dt.float32
F32R = mybir.dt.float32r

# Rowsum work split: chunks 0..RS_PE_CHUNKS-1 reduce on PE (ones-matmul),
# the rest accumulate elementwise on the otherwise-idle DVE and get folded
# in with one final ones-matmul.
RS_PE_CHUNKS = 64  # v1: all on PE; tune later


def _build_nc():
    nc = bacc.Bacc(
        "TRN2", target_bir_lowering=False, debug=False, num_devices=N_CORES
    )
    qT = nc.dram_tensor("qT", [D, HW], F32, kind="ExternalInput").ap()
    kT = nc.dram_tensor("kT", [D, N], F32, kind="ExternalInput").ap()
    vt = nc.dram_tensor("vt", [D, N], F32, kind="ExternalInput").ap()
    ones = nc.dram_tensor("ones", [D, 1], F32, kind="ExternalInput").ap()
    oT = nc.dram_tensor("oT", [D, HW], F32, kind="ExternalOutput").ap()

    with tile.TileContext(nc) as tc:
        with (
            tc.tile_pool(name="persist", bufs=1) as persist,
            tc.tile_pool(name="etile", bufs=6) as epool,
            tc.tile_pool(name="spsum", bufs=2, space="PSUM") as spsum,
            tc.tile_pool(name="apsum", bufs=1, space="PSUM") as apsum,
        ):
            qT_sb = persist.tile([D, HW], F32R, tag="qT_sb")
            ones_sb = persist.tile([D, 1], F32R, tag="ones_sb")
            kT_sb = [persist.tile([D, HW], F32R, tag=f"kT{i}", name=f"kT_sb{i}") for i in range(8)]
            vt_sb = [persist.tile([D, HW], F32R, tag=f"vt{i}", name=f"vt_sb{i}") for i in range(8)]

            nc.sync.dma_start(out=qT_sb[:], in_=qT[:].bitcast(F32R))
            nc.sync.dma_start(out=ones_sb[:], in_=ones[:].bitcast(F32R))
            # Interleave K/V pieces so PV(c) never waits behind the whole
            # K stream.
            for i in range(8):
                nc.sync.dma_start(out=kT_sb[i][:], in_=kT[:, i * HW : (i + 1) * HW].bitcast(F32R))
                nc.sync.dma_start(out=vt_sb[i][:], in_=vt[:, i * HW : (i + 1) * HW].bitcast(F32R))

            o_psum = apsum.tile([D, HW], F32, tag="o_psum")
            rs_psum = apsum.tile([1, HW], F32, tag="rs_psum")

            rs_acc = persist.tile([D, HW], F32, tag="rs_acc")
            nc.vector.memset(rs_acc[:], 0.0)

            def emit_qk(c):
                blk, off = c // 8, (c % 8) * 128
                s_ps = spsum.tile([D, HW], F32, tag="s_ps", name=f"s_ps{c}")
                for h in range(2):
                    nc.tensor.matmul(
                        s_ps[:, h * 512 : (h + 1) * 512],
                        kT_sb[blk][:, off : off + 128],
                        qT_sb[:, h * 512 : (h + 1) * 512],
                        start=True,
                        stop=True,
                    )
                return s_ps

            # Software-pipelined by one chunk: PE's program order is
            # QK(c+1) -> PV(c), so PE streams QK(c+1) while ACT exps S(c)
            # instead of stalling in-order behind PV(c)'s wait.
            s_tiles = {0: emit_qk(0)}
            first_pe_rs = True
            for c in range(NCHUNK):
                if c + 1 < NCHUNK:
                    s_tiles[c + 1] = emit_qk(c + 1)

                e_sb = epool.tile([D, HW], F32R, tag="e_sb", name=f"e_sb{c}")
                nc.scalar.activation(
                    e_sb[:],
                    s_tiles.pop(c)[:],
                    mybir.ActivationFunctionType.Exp,
                    scale=SCALE,
                )

                blk, off = c // 8, (c % 8) * 128
                for h in range(2):
                    nc.tensor.matmul(
                        o_psum[:, h * 512 : (h + 1) * 512],
                        vt_sb[blk][:, off : off + 128],
                        e_sb[:, h * 512 : (h + 1) * 512],
                        start=(c == 0),
                        stop=(c == NCHUNK - 1),
                    )

                # Rowsum: ~1/5 of chunks reduce on PE (ones-matmul), the rest
                # accumulate elementwise on the otherwise-idle DVE; balanced
                # so ACT's exp stream stays the critical path.
                if c % 5 == 4:
                    for h in range(2):
                        nc.tensor.matmul(
                            rs_psum[:, h * 512 : (h + 1) * 512],
                            ones_sb[:],
                            e_sb[:, h * 512 : (h + 1) * 512],
                            start=first_pe_rs,
                            stop=False,
                        )
                    first_pe_rs = False
                else:
                    nc.vector.tensor_add(
                        rs_acc[:], rs_acc[:], e_sb[:].bitcast(F32)
                    )

            # Fold the DVE partial sums into the PSUM rowsum (via an f32r
            # copy so the fp32r matmul verifier sees a rounded producer).
            rs_acc_r = persist.tile([D, HW], F32R, tag="rs_acc_r")
            nc.scalar.copy(rs_acc_r[:], rs_acc[:])
            for h in range(2):
                nc.tensor.matmul(
                    rs_psum[:, h * 512 : (h + 1) * 512],
                    ones_sb[:],
                    rs_acc_r[:, h * 512 : (h + 1) * 512],
                    start=first_pe_rs,
                    stop=True,
                )

            # softmax denominator -> reciprocal -> scale columns of o_psum
            rs_sb = persist.tile([1, HW], F32, tag="rs_sb")
            nc.scalar.copy(rs_sb[:], rs_psum[:])
            recip_sb = persist.tile([1, HW], F32, tag="recip_sb")
            nc.vector.reciprocal(recip_sb[:], rs_sb[:])
            bc_sb = persist.tile([D, HW], F32, tag="bc_sb")
            nc.gpsimd.partition_broadcast(bc_sb[:], recip_sb[:])
            o_sb = persist.tile([D, HW], F32, tag="o_sb")
            nc.vector.tensor_mul(o_sb[:], o_psum[:], bc_sb[:])
            nc.sync.dma_start(out=oT[:], in_=o_sb[:])

    nc.compile()
    return nc


_NC_CACHE = None


def _get_nc():
    global _NC_CACHE
    if _NC_CACHE is None:
        _NC_CACHE = _build_nc()
    return _NC_CACHE


def kernel(x: np.ndarray) -> np.ndarray:
    assert x.shape == (B, C, H, W), x.shape
    x = np.ascontiguousarray(x, dtype=np.float32)
    xr = x.reshape(B, C, HW)

    # K channel-major over all tokens: kT[d, b*1024+hw] = x[b, 128+d, hw]
    kT = np.ascontiguousarray(xr[:, 128:256, :].transpose(1, 0, 2)).reshape(D, N)
    # V chunk-transposed: vt[p, 128*j + v] = V[128*j + p, v],
    # V[n, v] = x[b, v, hw] with n = b*1024 + hw
    v_tok = np.ascontiguousarray(xr[:, 0:128, :].transpose(0, 2, 1)).reshape(N, D)
    vt = np.ascontiguousarray(v_tok.reshape(NCHUNK, 128, D).transpose(1, 0, 2)).reshape(
        D, N
    )

    ones_col = np.ones((D, 1), dtype=np.float32)
    in_maps = []
    for c in range(N_CORES):
        qT = np.ascontiguousarray(xr[c, 256:384, :])
        in_maps.append({"qT": qT, "kT": kT, "vt": vt, "ones": ones_col})

    nc = _get_nc()
    res = run_bass_kernel_spmd(nc, in_maps, list(range(N_CORES)))

    out = np.empty((B, D, H, W), dtype=np.float32)
    for c in range(N_CORES):
        out[c] = res.results[c]["oT"].reshape(D, H, W)
    return out



# revision 8
# speedup vs baseline: 1.1680x; 1.1680x over previous
"""Global-attention kernel for [8, 384, 32, 32] ConvAttention on 8 trn2 cores.

Math (per reference): tokens over B*H*W = 8192 positions, C = 384 channels
split as V/K/Q of 128 each; out = softmax(Q K^T / sqrt(128)) V, re-laid as
[B, 128, H, W].

Sharding: core c owns the 1024 query tokens of batch c (token n = b*1024+hw,
so batch == contiguous token block). K/V are replicated. Each core computes
its row block of the attention entirely locally; no collectives.

v2 design (vs v1): everything on-chip is fp16 (matmul rate identical to
f32r in the cost model, but the DVE gets 2-byte 2x mode for the rowsum and
all DMA halves). Work is split into 128 half-chunks (kv-chunk c, q-half h);
exp runs on ACT at a 1536-column granule (3 PSUM banks x 2 buffers + 2
banks for the output accumulator = all 8 banks), which amortizes the
per-instruction ACT overhead. The rowsum is entirely off the PE: DVE
accumulates exp tiles in fp16 and a final ones-matmul folds partitions.
Softmax reciprocal is broadcast via a tiny 1-partition PE matmul.
"""

import math

import numpy as np

import concourse.bass as bass
import concourse.tile as tile
from concourse import bacc, mybir
from concourse.bass_utils import run_bass_kernel_spmd

N_CORES = 8
B, C, H, W = 8, 384, 32, 32
HW = H * W            # 1024 tokens per batch == per core
N = B * HW            # 8192 total tokens
D = 128               # key/value width
NCHUNK = N // 128     # 64 kv chunks of 128 tokens
NHALF = 2 * NCHUNK    # 128 half-chunk work items (kv chunk, q half)
SLOTS = 3             # half-chunks per exp tile
NTILE = (NHALF + SLOTS - 1) // SLOTS  # 43 exp tiles (last has 2 slots)
SCALE = 1.0 / math.sqrt(D)
F16 = mybir.dt.float16
F32 = mybir.dt.float32
F32R = mybir.dt.float32r
BF16 = mybir.dt.bfloat16


def _tile_slots(t):
    return range(SLOTS * t, min(SLOTS * (t + 1), NHALF))


def _build_nc():
    nc = bacc.Bacc(
        "TRN2", target_bir_lowering=False, debug=False, num_devices=N_CORES
    )
    qT = nc.dram_tensor("qT", [D, HW], F16, kind="ExternalInput").ap()
    kT = nc.dram_tensor("kT", [D, N], F16, kind="ExternalInput").ap()
    vt = nc.dram_tensor("vt", [D, N], BF16, kind="ExternalInput").ap()
    ones = nc.dram_tensor("ones", [D, 1], BF16, kind="ExternalInput").ap()
    ones_row = nc.dram_tensor("ones_row", [1, D], F16, kind="ExternalInput").ap()
    oT = nc.dram_tensor("oT", [D, HW], F16, kind="ExternalOutput").ap()

    with tile.TileContext(nc) as tc:
        with (
            tc.tile_pool(name="persist", bufs=1) as persist,
            tc.tile_pool(name="etile", bufs=4) as epool,
            tc.tile_pool(name="spsum", bufs=2, space="PSUM") as spsum,
            tc.tile_pool(name="apsum", bufs=1, space="PSUM") as apsum,
        ):
            qT_sb = persist.tile([D, HW], F16, tag="qT_sb")
            ones_sb = persist.tile([D, 1], BF16, tag="ones_sb")
            onesr_sb = persist.tile([1, D], F16, tag="onesr_sb")
            kT_sb = [persist.tile([D, HW], F16, tag=f"kT{i}", name=f"kT_sb{i}") for i in range(8)]
            vt_sb = [persist.tile([D, HW], BF16, tag=f"vt{i}", name=f"vt_sb{i}") for i in range(8)]

            nc.sync.dma_start(out=qT_sb[:], in_=qT[:])
            nc.sync.dma_start(out=ones_sb[:], in_=ones[:])
            nc.sync.dma_start(out=onesr_sb[:], in_=ones_row[:])
            # Interleave K/V pieces so PV(c) never waits behind the whole
            # K stream.
            for i in range(8):
                nc.sync.dma_start(out=kT_sb[i][:], in_=kT[:, i * HW : (i + 1) * HW])
                nc.sync.dma_start(out=vt_sb[i][:], in_=vt[:, i * HW : (i + 1) * HW])

            o_psum = apsum.tile([D, HW], F32, tag="o_psum")
            rs_acc = persist.tile([D, HW], BF16, tag="rs_acc")
            nc.vector.memset(rs_acc[:], 0.0)

            def emit_qk_tile(t):
                s_ps = spsum.tile([D, SLOTS * 512], F32, tag="s_ps", name=f"s_ps{t}")
                for j, k in enumerate(_tile_slots(t)):
                    c, h = k // 2, k % 2
                    blk, off = c // 8, (c % 8) * 128
                    nc.tensor.matmul(
                        s_ps[:, j * 512 : (j + 1) * 512],
                        kT_sb[blk][:, off : off + 128],
                        qT_sb[:, h * 512 : (h + 1) * 512],
                        start=True,
                        stop=True,
                    )
                return s_ps

            # Software-pipelined by one tile: PE streams QK(t+1) while ACT
            # exps tile t, then drains PV(t).
            s_tiles = {0: emit_qk_tile(0)}
            seen_half = set()
            for t in range(NTILE):
                if t + 1 < NTILE:
                    s_tiles[t + 1] = emit_qk_tile(t + 1)

                nslots = len(_tile_slots(t))
                e_sb = epool.tile([D, SLOTS * 512], BF16, tag="e_sb", name=f"e_sb{t}")
                nc.scalar.activation(
                    e_sb[:, : nslots * 512],
                    s_tiles.pop(t)[:, : nslots * 512],
                    mybir.ActivationFunctionType.Exp,
                    scale=SCALE,
                )

                for j, k in enumerate(_tile_slots(t)):
                    c, h = k // 2, k % 2
                    blk, off = c // 8, (c % 8) * 128
                    nc.tensor.matmul(
                        o_psum[:, h * 512 : (h + 1) * 512],
                        vt_sb[blk][:, off : off + 128],
                        e_sb[:, j * 512 : (j + 1) * 512],
                        start=(h not in seen_half),
                        stop=(k >= NHALF - 2),
                    )
                    seen_half.add(h)
                    # fp16 rowsum accumulation on the (2x-mode) DVE
                    with nc.allow_low_precision(
                        reason="64 positive partials; fp16 accum err ~1e-3"
                    ):
                        nc.vector.tensor_add(
                            rs_acc[:, h * 512 : (h + 1) * 512],
                            rs_acc[:, h * 512 : (h + 1) * 512],
                            e_sb[:, j * 512 : (j + 1) * 512],
                        )

            # softmax denominator: partition-fold via ones-matmul, then
            # reciprocal, broadcast back across partitions with a
            # 1-contraction matmul, scale, and store.
            rs_tile = spsum.tile([D, SLOTS * 512], F32, tag="s_ps", name="rs_ps")
            rs_psum = rs_tile[0:1, 0:HW]
            for h in range(2):
                nc.tensor.matmul(
                    rs_tile[0:1, h * 512 : (h + 1) * 512],
                    ones_sb[:],
                    rs_acc[:, h * 512 : (h + 1) * 512],
                    start=True,
                    stop=True,
                )
            recip_sb = persist.tile([1, HW], F32, tag="recip_sb")
            nc.vector.reciprocal(recip_sb[:], rs_psum)
            bc_sb = persist.tile([D, HW], F32, tag="bc_sb")
            nc.gpsimd.partition_broadcast(bc_sb[:], recip_sb[:])
            o_sb = persist.tile([D, HW], F16, tag="o_sb")
            nc.vector.tensor_mul(o_sb[:], o_psum[:], bc_sb[:])
            nc.sync.dma_start(out=oT[:], in_=o_sb[:])

    nc.compile()
    return nc


_NC_CACHE = None


def _get_nc():
    global _NC_CACHE
    if _NC_CACHE is None:
        _NC_CACHE = _build_nc()
    return _NC_CACHE


def prepare_in_maps(x: np.ndarray) -> list[dict]:
    xr = np.ascontiguousarray(x, dtype=np.float32).reshape(B, C, HW)

    # K channel-major over all tokens: kT[d, b*1024+hw] = x[b, 128+d, hw]
    kT = (
        np.ascontiguousarray(xr[:, 128:256, :].transpose(1, 0, 2))
        .reshape(D, N)
        .astype(np.float16)
    )
    # V chunk-transposed: vt[p, 128*j + v] = V[128*j + p, v],
    # V[n, v] = x[b, v, hw] with n = b*1024 + hw
    v_tok = np.ascontiguousarray(xr[:, 0:128, :].transpose(0, 2, 1)).reshape(N, D)
    import ml_dtypes

    vt = (
        np.ascontiguousarray(v_tok.reshape(NCHUNK, 128, D).transpose(1, 0, 2))
        .reshape(D, N)
        .astype(ml_dtypes.bfloat16)
    )

    ones_col = np.ones((D, 1), dtype=ml_dtypes.bfloat16)
    ones_row = np.ones((1, D), dtype=np.float16)
    in_maps = []
    for c in range(N_CORES):
        qT = np.ascontiguousarray(xr[c, 256:384, :]).astype(np.float16)
        in_maps.append(
            {"qT": qT, "kT": kT, "vt": vt, "ones": ones_col, "ones_row": ones_row}
        )
    return in_maps


def kernel(x: np.ndarray) -> np.ndarray:
    assert x.shape == (B, C, H, W), x.shape
    in_maps = prepare_in_maps(x)
    nc = _get_nc()
    res = run_bass_kernel_spmd(nc, in_maps, list(range(N_CORES)))

    out = np.empty((B, D, H, W), dtype=np.float32)
    for c in range(N_CORES):
        out[c] = res.results[c]["oT"].astype(np.float32).reshape(D, H, W)
    return out


# revision 9
# speedup vs baseline: 1.2248x; 1.0486x over previous
"""Global-attention kernel for [8, 384, 32, 32] ConvAttention on 8 trn2 cores.

Math (per reference): tokens over B*H*W = 8192 positions, C = 384 channels
split as V/K/Q of 128 each; out = softmax(Q K^T / sqrt(128)) V, re-laid as
[B, 128, H, W].

Sharding: core c owns the 1024 query tokens of batch c (token n = b*1024+hw,
so batch == contiguous token block). K/V are replicated. Each core computes
its row block of the attention entirely locally; no collectives.

v2 design (vs v1): everything on-chip is fp16 (matmul rate identical to
f32r in the cost model, but the DVE gets 2-byte 2x mode for the rowsum and
all DMA halves). Work is split into 128 half-chunks (kv-chunk c, q-half h);
exp runs on ACT at a 1536-column granule (3 PSUM banks x 2 buffers + 2
banks for the output accumulator = all 8 banks), which amortizes the
per-instruction ACT overhead. The rowsum is entirely off the PE: DVE
accumulates exp tiles in fp16 and a final ones-matmul folds partitions.
Softmax reciprocal is broadcast via a tiny 1-partition PE matmul.
"""

import math

import numpy as np

import concourse.bass as bass
import concourse.tile as tile
from concourse import bacc, mybir
from concourse.bass_utils import run_bass_kernel_spmd

N_CORES = 8
B, C, H, W = 8, 384, 32, 32
HW = H * W            # 1024 tokens per batch == per core
N = B * HW            # 8192 total tokens
D = 128               # key/value width
NCHUNK = N // 128     # 64 kv chunks of 128 tokens
NHALF = 2 * NCHUNK    # 128 half-chunk work items (kv chunk, q half)
SLOTS = 3             # half-chunks per exp tile
NTILE = (NHALF + SLOTS - 1) // SLOTS  # 43 exp tiles (last has 2 slots)
SCALE = 1.0 / math.sqrt(D)
F16 = mybir.dt.float16
F32 = mybir.dt.float32
F32R = mybir.dt.float32r
BF16 = mybir.dt.bfloat16


def _tile_slots(t):
    return range(SLOTS * t, min(SLOTS * (t + 1), NHALF))


def _build_nc():
    nc = bacc.Bacc(
        "TRN2", target_bir_lowering=False, debug=False, num_devices=N_CORES
    )
    qT = nc.dram_tensor("qT", [D, HW], F16, kind="ExternalInput").ap()
    kT = nc.dram_tensor("kT", [D, N], F16, kind="ExternalInput").ap()
    vt = nc.dram_tensor("vt", [D, N], BF16, kind="ExternalInput").ap()
    ones = nc.dram_tensor("ones", [D, 1], BF16, kind="ExternalInput").ap()
    ones_row = nc.dram_tensor("ones_row", [1, D], F16, kind="ExternalInput").ap()
    oT = nc.dram_tensor("oT", [D, HW], F16, kind="ExternalOutput").ap()

    with tile.TileContext(nc) as tc:
        with (
            tc.tile_pool(name="persist", bufs=1) as persist,
            tc.tile_pool(name="etile", bufs=4) as epool,
            tc.tile_pool(name="spsum", bufs=2, space="PSUM") as spsum,
            tc.tile_pool(name="apsum", bufs=1, space="PSUM") as apsum,
        ):
            qT_sb = persist.tile([D, HW], F16, tag="qT_sb")
            ones_sb = persist.tile([D, 1], BF16, tag="ones_sb")
            onesr_sb = persist.tile([1, D], F16, tag="onesr_sb")
            kT_sb = [persist.tile([D, HW], F16, tag=f"kT{i}", name=f"kT_sb{i}") for i in range(8)]
            vt_sb = [persist.tile([D, HW], BF16, tag=f"vt{i}", name=f"vt_sb{i}") for i in range(8)]

            # Load order tuned for pipeline startup: the first QK tile only
            # needs kT chunks 0-1 and qT, so land those first (fine-grained
            # first slice), then stream the rest interleaved.
            nc.sync.dma_start(out=kT_sb[0][:, 0:256], in_=kT[:, 0:256])
            nc.sync.dma_start(out=qT_sb[:], in_=qT[:])
            nc.sync.dma_start(out=ones_sb[:], in_=ones[:])
            # Warm the exp table during the input stream so the ~1.3us
            # ACT_TABLE_LOAD doesn't delay exp(0).
            warm_sb = persist.tile([D, 1], BF16, tag="warm_sb")
            nc.scalar.activation(
                warm_sb[:], ones_sb[:], mybir.ActivationFunctionType.Exp, scale=1.0
            )
            nc.sync.dma_start(out=kT_sb[0][:, 256:HW], in_=kT[:, 256:HW])
            nc.sync.dma_start(out=vt_sb[0][:], in_=vt[:, 0:HW])
            nc.sync.dma_start(out=onesr_sb[:], in_=ones_row[:])
            for i in range(1, 8):
                nc.sync.dma_start(out=kT_sb[i][:], in_=kT[:, i * HW : (i + 1) * HW])
                nc.sync.dma_start(out=vt_sb[i][:], in_=vt[:, i * HW : (i + 1) * HW])

            o_psum = apsum.tile([D, HW], F32, tag="o_psum")
            rs_acc = persist.tile([D, HW], BF16, tag="rs_acc")
            nc.vector.memset(rs_acc[:], 0.0)

            def emit_qk_tile(t):
                s_ps = spsum.tile([D, SLOTS * 512], F32, tag="s_ps", name=f"s_ps{t}")
                for j, k in enumerate(_tile_slots(t)):
                    c, h = k // 2, k % 2
                    blk, off = c // 8, (c % 8) * 128
                    nc.tensor.matmul(
                        s_ps[:, j * 512 : (j + 1) * 512],
                        kT_sb[blk][:, off : off + 128],
                        qT_sb[:, h * 512 : (h + 1) * 512],
                        start=True,
                        stop=True,
                    )
                return s_ps

            # Software-pipelined by one tile: PE streams QK(t+1) while ACT
            # exps tile t, then drains PV(t).
            s_tiles = {0: emit_qk_tile(0)}
            seen_half = set()
            for t in range(NTILE):
                if t + 1 < NTILE:
                    s_tiles[t + 1] = emit_qk_tile(t + 1)

                nslots = len(_tile_slots(t))
                e_sb = epool.tile([D, SLOTS * 512], BF16, tag="e_sb", name=f"e_sb{t}")
                nc.scalar.activation(
                    e_sb[:, : nslots * 512],
                    s_tiles.pop(t)[:, : nslots * 512],
                    mybir.ActivationFunctionType.Exp,
                    scale=SCALE,
                )

                for j, k in enumerate(_tile_slots(t)):
                    c, h = k // 2, k % 2
                    blk, off = c // 8, (c % 8) * 128
                    nc.tensor.matmul(
                        o_psum[:, h * 512 : (h + 1) * 512],
                        vt_sb[blk][:, off : off + 128],
                        e_sb[:, j * 512 : (j + 1) * 512],
                        start=(h not in seen_half),
                        stop=(k >= NHALF - 2),
                    )
                    seen_half.add(h)
                    # fp16 rowsum accumulation on the (2x-mode) DVE
                    with nc.allow_low_precision(
                        reason="64 positive partials; fp16 accum err ~1e-3"
                    ):
                        nc.vector.tensor_add(
                            rs_acc[:, h * 512 : (h + 1) * 512],
                            rs_acc[:, h * 512 : (h + 1) * 512],
                            e_sb[:, j * 512 : (j + 1) * 512],
                        )

            # softmax denominator: partition-fold via ones-matmul, then
            # reciprocal, broadcast back across partitions with a
            # 1-contraction matmul, scale, and store.
            rs_tile = spsum.tile([D, SLOTS * 512], F32, tag="s_ps", name="rs_ps")
            rs_psum = rs_tile[0:1, 0:HW]
            for h in range(2):
                nc.tensor.matmul(
                    rs_tile[0:1, h * 512 : (h + 1) * 512],
                    ones_sb[:],
                    rs_acc[:, h * 512 : (h + 1) * 512],
                    start=True,
                    stop=True,
                )
            recip_sb = persist.tile([1, HW], F32, tag="recip_sb")
            nc.vector.reciprocal(recip_sb[:], rs_psum)
            bc_sb = persist.tile([D, HW], F32, tag="bc_sb")
            nc.gpsimd.partition_broadcast(bc_sb[:], recip_sb[:])
            o_sb = persist.tile([D, HW], F16, tag="o_sb")
            nc.vector.tensor_mul(o_sb[:], o_psum[:], bc_sb[:])
            nc.sync.dma_start(out=oT[:], in_=o_sb[:])

    nc.compile()
    return nc


_NC_CACHE = None


def _get_nc():
    global _NC_CACHE
    if _NC_CACHE is None:
        _NC_CACHE = _build_nc()
    return _NC_CACHE


def prepare_in_maps(x: np.ndarray) -> list[dict]:
    xr = np.ascontiguousarray(x, dtype=np.float32).reshape(B, C, HW)

    # K channel-major over all tokens: kT[d, b*1024+hw] = x[b, 128+d, hw]
    kT = (
        np.ascontiguousarray(xr[:, 128:256, :].transpose(1, 0, 2))
        .reshape(D, N)
        .astype(np.float16)
    )
    # V chunk-transposed: vt[p, 128*j + v] = V[128*j + p, v],
    # V[n, v] = x[b, v, hw] with n = b*1024 + hw
    v_tok = np.ascontiguousarray(xr[:, 0:128, :].transpose(0, 2, 1)).reshape(N, D)
    import ml_dtypes

    vt = (
        np.ascontiguousarray(v_tok.reshape(NCHUNK, 128, D).transpose(1, 0, 2))
        .reshape(D, N)
        .astype(ml_dtypes.bfloat16)
    )

    ones_col = np.ones((D, 1), dtype=ml_dtypes.bfloat16)
    ones_row = np.ones((1, D), dtype=np.float16)
    in_maps = []
    for c in range(N_CORES):
        qT = np.ascontiguousarray(xr[c, 256:384, :]).astype(np.float16)
        in_maps.append(
            {"qT": qT, "kT": kT, "vt": vt, "ones": ones_col, "ones_row": ones_row}
        )
    return in_maps


def kernel(x: np.ndarray) -> np.ndarray:
    assert x.shape == (B, C, H, W), x.shape
    in_maps = prepare_in_maps(x)
    nc = _get_nc()
    res = run_bass_kernel_spmd(nc, in_maps, list(range(N_CORES)))

    out = np.empty((B, D, H, W), dtype=np.float32)
    for c in range(N_CORES):
        out[c] = res.results[c]["oT"].astype(np.float32).reshape(D, H, W)
    return out


# revision 10
# speedup vs baseline: 1.2496x; 1.0203x over previous
"""Global-attention kernel for [8, 384, 32, 32] ConvAttention on 8 trn2 cores.

Math (per reference): tokens over B*H*W = 8192 positions, C = 384 channels
split as V/K/Q of 128 each; out = softmax(Q K^T / sqrt(128)) V, re-laid as
[B, 128, H, W].

Sharding: core c owns the 1024 query tokens of batch c (token n = b*1024+hw,
so batch == contiguous token block). K/V are replicated. Each core computes
its row block of the attention entirely locally; no collectives.

v2 design (vs v1): everything on-chip is fp16 (matmul rate identical to
f32r in the cost model, but the DVE gets 2-byte 2x mode for the rowsum and
all DMA halves). Work is split into 128 half-chunks (kv-chunk c, q-half h);
exp runs on ACT at a 1536-column granule (3 PSUM banks x 2 buffers + 2
banks for the output accumulator = all 8 banks), which amortizes the
per-instruction ACT overhead. The rowsum is entirely off the PE: DVE
accumulates exp tiles in fp16 and a final ones-matmul folds partitions.
Softmax reciprocal is broadcast via a tiny 1-partition PE matmul.
"""

import math

import numpy as np

import concourse.bass as bass
import concourse.tile as tile
from concourse import bacc, mybir
from concourse.bass_utils import run_bass_kernel_spmd

N_CORES = 8
B, C, H, W = 8, 384, 32, 32
HW = H * W            # 1024 tokens per batch == per core
N = B * HW            # 8192 total tokens
D = 128               # key/value width
NCHUNK = N // 128     # 64 kv chunks of 128 tokens
NHALF = 2 * NCHUNK    # 128 half-chunk work items (kv chunk, q half)
SLOTS = 3             # half-chunks per exp tile
NTILE = (NHALF + SLOTS - 1) // SLOTS  # 43 exp tiles (last has 2 slots)
SCALE = 1.0 / math.sqrt(D)
F16 = mybir.dt.float16
F32 = mybir.dt.float32
F32R = mybir.dt.float32r
BF16 = mybir.dt.bfloat16


def _tile_slots(t):
    return range(SLOTS * t, min(SLOTS * (t + 1), NHALF))


def _build_nc():
    nc = bacc.Bacc(
        "TRN2", target_bir_lowering=False, debug=False, num_devices=N_CORES
    )
    qT = nc.dram_tensor("qT", [D, HW], F16, kind="ExternalInput").ap()
    kT = nc.dram_tensor("kT", [D, N], F16, kind="ExternalInput").ap()
    vt = nc.dram_tensor("vt", [D, N], BF16, kind="ExternalInput").ap()
    ones = nc.dram_tensor("ones", [D, 1], BF16, kind="ExternalInput").ap()
    ones_row = nc.dram_tensor("ones_row", [1, D], F16, kind="ExternalInput").ap()
    oT = nc.dram_tensor("oT", [D, HW], F16, kind="ExternalOutput").ap()

    with tile.TileContext(nc) as tc:
        with (
            tc.tile_pool(name="persist", bufs=1) as persist,
            tc.tile_pool(name="etile", bufs=4) as epool,
            tc.tile_pool(name="spsum", bufs=2, space="PSUM") as spsum,
            tc.tile_pool(name="apsum", bufs=1, space="PSUM") as apsum,
        ):
            qT_sb = persist.tile([D, HW], F16, tag="qT_sb")
            ones_sb = persist.tile([D, 1], BF16, tag="ones_sb")
            onesr_sb = persist.tile([1, D], F16, tag="onesr_sb")
            kT_sb = [persist.tile([D, HW], F16, tag=f"kT{i}", name=f"kT_sb{i}") for i in range(8)]
            vt_sb = [persist.tile([D, HW], BF16, tag=f"vt{i}", name=f"vt_sb{i}") for i in range(8)]

            # Load order tuned for pipeline startup: the first QK tile only
            # needs kT chunks 0-1 and qT, so land those first (fine-grained
            # first slice), then stream the rest interleaved.
            nc.sync.dma_start(out=kT_sb[0][:, 0:256], in_=kT[:, 0:256])
            nc.sync.dma_start(out=qT_sb[:], in_=qT[:])
            nc.sync.dma_start(out=ones_sb[:], in_=ones[:])
            # Warm the exp table during the input stream so the ~1.3us
            # ACT_TABLE_LOAD doesn't delay exp(0).
            warm_sb = persist.tile([D, 1], BF16, tag="warm_sb")
            nc.scalar.activation(
                warm_sb[:], ones_sb[:], mybir.ActivationFunctionType.Exp, scale=1.0
            )
            nc.sync.dma_start(out=kT_sb[0][:, 256:HW], in_=kT[:, 256:HW])
            nc.sync.dma_start(out=vt_sb[0][:], in_=vt[:, 0:HW])
            nc.sync.dma_start(out=onesr_sb[:], in_=ones_row[:])
            for i in range(1, 8):
                nc.sync.dma_start(out=kT_sb[i][:], in_=kT[:, i * HW : (i + 1) * HW])
                nc.sync.dma_start(out=vt_sb[i][:], in_=vt[:, i * HW : (i + 1) * HW])

            o_psum = apsum.tile([D, HW], F32, tag="o_psum")
            rs_acc = persist.tile([D, HW], BF16, tag="rs_acc")
            nc.vector.memset(rs_acc[:], 0.0)

            def emit_qk_tile(t):
                s_ps = spsum.tile([D, SLOTS * 512], F32, tag="s_ps", name=f"s_ps{t}")
                for j, k in enumerate(_tile_slots(t)):
                    c, h = k // 2, k % 2
                    blk, off = c // 8, (c % 8) * 128
                    nc.tensor.matmul(
                        s_ps[:, j * 512 : (j + 1) * 512],
                        kT_sb[blk][:, off : off + 128],
                        qT_sb[:, h * 512 : (h + 1) * 512],
                        start=True,
                        stop=True,
                    )
                return s_ps

            # Software-pipelined by one tile: PE streams QK(t+1) while ACT
            # exps tile t, then drains PV(t).
            s_tiles = {0: emit_qk_tile(0)}
            seen_half = set()
            for t in range(NTILE):
                if t + 1 < NTILE:
                    s_tiles[t + 1] = emit_qk_tile(t + 1)

                nslots = len(_tile_slots(t))
                e_sb = epool.tile([D, SLOTS * 512], BF16, tag="e_sb", name=f"e_sb{t}")
                nc.scalar.activation(
                    e_sb[:, : nslots * 512],
                    s_tiles.pop(t)[:, : nslots * 512],
                    mybir.ActivationFunctionType.Exp,
                    scale=SCALE,
                )

                for j, k in enumerate(_tile_slots(t)):
                    c, h = k // 2, k % 2
                    blk, off = c // 8, (c % 8) * 128
                    nc.tensor.matmul(
                        o_psum[:, h * 512 : (h + 1) * 512],
                        vt_sb[blk][:, off : off + 128],
                        e_sb[:, j * 512 : (j + 1) * 512],
                        start=(h not in seen_half),
                        stop=(k >= NHALF - 2),
                    )
                    seen_half.add(h)
                    # fp16 rowsum accumulation on the (2x-mode) DVE
                    with nc.allow_low_precision(
                        reason="64 positive partials; fp16 accum err ~1e-3"
                    ):
                        nc.vector.tensor_add(
                            rs_acc[:, h * 512 : (h + 1) * 512],
                            rs_acc[:, h * 512 : (h + 1) * 512],
                            e_sb[:, j * 512 : (j + 1) * 512],
                        )

            # softmax denominator: partition-fold via ones-matmul, then
            # reciprocal, broadcast back across partitions with a
            # 1-contraction matmul, scale, and store.
            # Pipelined by q-half across PE -> DVE -> Pool -> DVE -> DMA so
            # the h=1 chain hides behind h=0's downstream stages.
            rs_tile = spsum.tile([D, SLOTS * 512], F32, tag="s_ps", name="rs_ps")
            recip_sb = persist.tile([1, HW], F32, tag="recip_sb")
            bc_sb = persist.tile([D, HW], F32, tag="bc_sb")
            o_sb = persist.tile([D, HW], F16, tag="o_sb")

            def sl(ap, h):
                return ap[:, h * 512 : (h + 1) * 512]

            for h in range(2):
                nc.tensor.matmul(
                    rs_tile[0:1, h * 512 : (h + 1) * 512],
                    ones_sb[:],
                    sl(rs_acc, h),
                    start=True,
                    stop=True,
                )
            nc.vector.reciprocal(
                recip_sb[:, 0:512], rs_tile[0:1, 0:512]
            )
            nc.gpsimd.partition_broadcast(sl(bc_sb, 0), recip_sb[:, 0:512])
            nc.vector.reciprocal(
                recip_sb[:, 512:HW], rs_tile[0:1, 512:HW]
            )
            nc.vector.tensor_mul(sl(o_sb, 0), sl(o_psum, 0), sl(bc_sb, 0))
            nc.gpsimd.partition_broadcast(sl(bc_sb, 1), recip_sb[:, 512:HW])
            nc.sync.dma_start(out=sl(oT, 0), in_=sl(o_sb, 0))
            nc.vector.tensor_mul(sl(o_sb, 1), sl(o_psum, 1), sl(bc_sb, 1))
            nc.sync.dma_start(out=sl(oT, 1), in_=sl(o_sb, 1))

    nc.compile()
    return nc


_NC_CACHE = None


def _get_nc():
    global _NC_CACHE
    if _NC_CACHE is None:
        _NC_CACHE = _build_nc()
    return _NC_CACHE


def prepare_in_maps(x: np.ndarray) -> list[dict]:
    xr = np.ascontiguousarray(x, dtype=np.float32).reshape(B, C, HW)

    # K channel-major over all tokens: kT[d, b*1024+hw] = x[b, 128+d, hw]
    kT = (
        np.ascontiguousarray(xr[:, 128:256, :].transpose(1, 0, 2))
        .reshape(D, N)
        .astype(np.float16)
    )
    # V chunk-transposed: vt[p, 128*j + v] = V[128*j + p, v],
    # V[n, v] = x[b, v, hw] with n = b*1024 + hw
    v_tok = np.ascontiguousarray(xr[:, 0:128, :].transpose(0, 2, 1)).reshape(N, D)
    import ml_dtypes

    vt = (
        np.ascontiguousarray(v_tok.reshape(NCHUNK, 128, D).transpose(1, 0, 2))
        .reshape(D, N)
        .astype(ml_dtypes.bfloat16)
    )

    ones_col = np.ones((D, 1), dtype=ml_dtypes.bfloat16)
    ones_row = np.ones((1, D), dtype=np.float16)
    in_maps = []
    for c in range(N_CORES):
        qT = np.ascontiguousarray(xr[c, 256:384, :]).astype(np.float16)
        in_maps.append(
            {"qT": qT, "kT": kT, "vt": vt, "ones": ones_col, "ones_row": ones_row}
        )
    return in_maps


def kernel(x: np.ndarray) -> np.ndarray:
    assert x.shape == (B, C, H, W), x.shape
    in_maps = prepare_in_maps(x)
    nc = _get_nc()
    res = run_bass_kernel_spmd(nc, in_maps, list(range(N_CORES)))

    out = np.empty((B, D, H, W), dtype=np.float32)
    for c in range(N_CORES):
        out[c] = res.results[c]["oT"].astype(np.float32).reshape(D, H, W)
    return out
